# revision 1
# baseline (speedup 1.0000x reference)
"""DigitCaps dynamic-routing kernel for 8 Trainium2 NeuronCores.

Problem: nn_DigitCaps  (B=128, R=1152, C=10, O=16, I=338, 3 routing iters)

    u_hat[b,r,c,o] = sum_i W[r,c,o,i] * x[b,r,i]
    b_ij = 0
    for it in 3:
        c_ij = softmax(b_ij, axis=routes)
        s[b,c,o] = sum_r c_ij[r,c] * u_hat[b,r,c,o]
        v = squash(s) = s*|s|/(1+s^2)
        if it < 2: b_ij += mean_b sum_o u_hat*v

Sharding: routes (R) split 8 ways -> 144 routes/core.  Each core computes
u_hat for its routes (PE matmuls, batch=128 on partitions), keeps it fully
SBUF-resident, and runs the routing iterations locally.  The only cross-core
communication is one fused AllReduce per iteration carrying the partial
softmax-weighted sum s (numerator) and the softmax denominator d.

Host-side prep (inside kernel()): per-core shards are transposed so both
matmul operands arrive with the contraction dim (i) on SBUF partitions:
  xk[r, i, b]   (i padded 338->384)   lhsT tiles [i=128, b=128]
  wk[r, i, o, c] (i padded 338->384)  rhs  tiles [i=128, (o,c)=160]
"""

import numpy as np

B, R, C, O, I = 128, 1152, 10, 16, 338
N_CORES = 8
R_LOC = R // N_CORES          # 144
IP = 384                      # i padded to 3*128
NCH = IP // 128               # 3 contraction chunks
CO = C * O                    # 160
CHUNKS = (128, 128, 82)       # unpadded i-chunk sizes (sum = I)
F32 = None                    # set lazily (mybir.dt.float32)

_CACHE = {}


def _build_nc(r_loc=R_LOC, n_cores=N_CORES, stages=99, ws_parts=4, reps=1,
              stream_bufs=6, dma_group=1, s0_late=False, host_layout=1,
              no_cc=False, spass_split=False):
    import concourse.tile as tile
    from concourse import bacc, mybir

    f32 = mybir.dt.float32
    nc = bacc.Bacc("TRN2", target_bir_lowering=False, debug=False,
                   enable_asserts=False, num_devices=n_cores)

    if host_layout == 3:
        ng = r_loc // dma_group
        xk = nc.dram_tensor("xk", [ng * I * dma_group * B], f32,
                            kind="ExternalInput")
        wk = nc.dram_tensor("wk", [ng * I * dma_group * CO], f32,
                            kind="ExternalInput")
    elif host_layout == 2:
        xk = nc.dram_tensor("xk", [r_loc // dma_group, 128, dma_group, NCH, B],
                            f32, kind="ExternalInput")
        wk = nc.dram_tensor("wk", [r_loc // dma_group, 128, dma_group, NCH, CO],
                            f32, kind="ExternalInput")
    else:
        xk = nc.dram_tensor("xk", [r_loc, NCH, 128, B], f32, kind="ExternalInput")
        wk = nc.dram_tensor("wk", [r_loc, NCH, 128, CO], f32, kind="ExternalInput")
    out = nc.dram_tensor("out", [B, CO], f32, kind="ExternalOutput")

    groups = [list(range(n_cores))]

    with tile.TileContext(nc) as tc:
        with (
            tc.tile_pool(name="u", bufs=1) as u_pool,
            tc.tile_pool(name="stream", bufs=stream_bufs) as stream,
            tc.tile_pool(name="small", bufs=1) as small,
            tc.tile_pool(name="scratch", bufs=4) as scratch_pool,
            tc.tile_pool(name="upsum", bufs=4, space="PSUM") as upsum_pool,
            tc.tile_pool(name="apsum", bufs=2, space="PSUM") as apsum_pool,
            tc.tile_pool(name="wbc", bufs=2, space="PSUM") as wbc_pool,
            tc.tile_pool(name="dram", bufs=1, space="DRAM") as dram,
        ):
            # u_hat resident tile: [b=128, o, c, r]
            u_sb = u_pool.tile([B, O, C, r_loc], f32)
            s0_acc = small.tile([B, CO], f32)       # (o,c) flat
            nc.vector.memset(s0_acc[:], 0.0)
            ones_sb = small.tile([1, 128], f32)
            nc.vector.memset(ones_sb[:], 1.0)
            b_sb = small.tile([1, C, r_loc], f32)
            nc.vector.memset(b_sb[:], 0.0)
            b_shift = small.tile([1, C, r_loc], f32)
            ones_r = small.tile([B, r_loc], f32)
            nc.vector.memset(ones_r[:], 1.0)
            w_sb = small.tile([B, C, r_loc], f32)   # exp(b) broadcast to all b
            s_sb = small.tile([B, CO], f32)         # s numerator / s (o,c) flat
            v_sb = small.tile([B, CO], f32)
            d_all = small.tile([B, C], f32)
            rd_sb = small.tile([B, C], f32)
            d_row = small.tile([1, CO], f32)
            mx_row = small.tile([1, C], f32)
            sq_t = small.tile([B, CO], f32)
            den_t = small.tile([B, CO], f32)
            rden_t = small.tile([B, CO], f32)
            sabs_t = small.tile([B, CO], f32)
            st_t = small.tile([B, CO], f32)
            out_sb = small.tile([B, CO], f32)

            for rep in range(reps):
                if rep > 0:
                    nc.vector.memset(s0_acc[:], 0.0)
                    nc.vector.memset(b_sb[:], 0.0)
                # ---------- Phase A: u_hat = x @ W per route ----------
                G = dma_group
                assert r_loc % G == 0
                for rg in range(r_loc // G):
                    x_t = stream.tile([128, G, NCH, B], f32, tag="x")
                    w_t = stream.tile([128, G, NCH, CO], f32, tag="w")
                    if host_layout == 3:
                        xoff = rg * I * G * B
                        woff = rg * I * G * CO
                        for ch, K in enumerate(CHUNKS):
                            nc.sync.dma_start(
                                x_t[0:K, :, ch, :],
                                xk[xoff:xoff + K * G * B].rearrange(
                                    "(p g b) -> p g b", p=K, g=G))
                            xoff += K * G * B
                            nc.sync.dma_start(
                                w_t[0:K, :, ch, :],
                                wk[woff:woff + K * G * CO].rearrange(
                                    "(p g n) -> p g n", p=K, g=G))
                            woff += K * G * CO
                    elif host_layout == 2:
                        nc.sync.dma_start(x_t[:], xk[rg])
                        nc.sync.dma_start(w_t[:], wk[rg])
                    else:
                        nc.sync.dma_start(
                            x_t[:], xk[rg * G:(rg + 1) * G].rearrange(
                                "g ch p b -> p g ch b"))
                        nc.sync.dma_start(
                            w_t[:], wk[rg * G:(rg + 1) * G].rearrange(
                                "g ch p n -> p g ch n"))
                    for g in range(G):
                        r = rg * G + g
                        upsum = upsum_pool.tile([B, CO], f32)
                        for ch in range(NCH):
                            K = CHUNKS[ch] if host_layout == 3 else 128
                            nc.tensor.matmul(upsum[:], x_t[0:K, g, ch, :],
                                             w_t[0:K, g, ch, :],
                                             start=(ch == 0), stop=(ch == NCH - 1))
                        # copy into resident u_hat ([b,(o,c)] -> [b,o,c,r])
                        nc.scalar.activation(
                            u_sb[:, :, :, r],
                            upsum[:].rearrange("p (o c) -> p o c", o=O),
                            mybir.ActivationFunctionType.Copy)
                        if not s0_late:
                            nc.vector.tensor_tensor(s0_acc[:], s0_acc[:], upsum[:],
                                                    mybir.AluOpType.add)
                if s0_late:
                    # s0 = sum_r u_hat via 16 reduces over the resident tile
                    for o in range(O):
                        nc.vector.tensor_reduce(
                            s0_acc[:, o * C:(o + 1) * C], u_sb[:, o, :, :],
                            mybir.AxisListType.X, mybir.AluOpType.add)

                # ---------- helpers ----------
                def all_reduce(sb_src, with_d, it):
                    rows = B + 1 if with_d else B
                    ar_in = dram.tile([rows, CO], f32, name=f"ar_in{rep}_{it}")
                    ar_out = dram.tile([rows, CO], f32, addr_space="Shared",
                                       name=f"ar_out{rep}_{it}")
                    nc.sync.dma_start(ar_in[0:B, :], sb_src[:])
                    if with_d:
                        nc.sync.dma_start(ar_in[B:B + 1, 0:C], d_all[0:1, :])
                    if no_cc:
                        nc.sync.dma_start(ar_out[:, :], ar_in[:, :])
                    else:
                        nc.gpsimd.collective_compute(
                            "AllReduce", mybir.AluOpType.add, replica_groups=groups,
                            ins=[ar_in.opt()], outs=[ar_out.opt()])
                    nc.sync.dma_start(s_sb[:], ar_out[0:B, :])
                    if with_d:
                        d2 = small.tile([1, C], f32, name=f"d2_{rep}_{it}")
                        nc.sync.dma_start(d2[:], ar_out[B:B + 1, 0:C])
                        dps = wbc_pool.tile([B, 512], f32, tag="wbc", name=f"dps{rep}_{it}")
                        nc.tensor.matmul(dps[:, 0:C], ones_sb[:], d2[:])
                        nc.vector.reciprocal(rd_sb[:], dps[:, 0:C])
                        for o in range(O):
                            nc.vector.tensor_tensor(
                                s_sb[:, o * C:(o + 1) * C], s_sb[:, o * C:(o + 1) * C],
                                rd_sb[:], mybir.AluOpType.mult)
                    else:
                        nc.vector.tensor_scalar_mul(s_sb[:], s_sb[:], 1.0 / (r_loc * n_cores))

                def squash():
                    # v = s*|s| / (1+s^2)
                    nc.vector.tensor_tensor(sq_t[:], s_sb[:], s_sb[:],
                                            mybir.AluOpType.mult)
                    nc.vector.tensor_scalar_add(den_t[:], sq_t[:], 1.0)
                    nc.vector.reciprocal(rden_t[:], den_t[:])
                    nc.scalar.activation(sabs_t[:], s_sb[:],
                                         mybir.ActivationFunctionType.Abs)
                    nc.vector.tensor_tensor(st_t[:], s_sb[:], sabs_t[:],
                                            mybir.AluOpType.mult)
                    nc.vector.tensor_tensor(v_sb[:], st_t[:], rden_t[:],
                                            mybir.AluOpType.mult)

                def agreement(it):
                    # b += (1/B) * sum_b sum_o u_hat * v   (PE, K=b contraction)
                    for c in range(C):
                        aps = apsum_pool.tile([1, r_loc], f32, tag="a", name=f"a{rep}_{it}_{c}")
                        for o in range(O):
                            j = o * C + c
                            nc.tensor.matmul(aps[:], v_sb[:, j:j + 1], u_sb[:, o, c, :],
                                             start=(o == 0), stop=(o == O - 1))
                        nc.vector.scalar_tensor_tensor(
                            b_sb[0:1, c, :], aps[:], 1.0 / B, b_sb[0:1, c, :],
                            mybir.AluOpType.mult, mybir.AluOpType.add)

                def weights_and_s(it):
                    if ws_parts < 4:
                        # truncated profiling builds: keep AR operands written
                        nc.vector.memset(s_sb[:], 0.0)
                        nc.vector.memset(w_sb[:], 0.0)
                        nc.vector.memset(d_all[:], 0.0)
                    if ws_parts < 1:
                        return
                    # LOCAL softmax max-subtraction; the cross-core max arrives
                    # via a small AllReduce(max) that overlaps exp + the s-pass,
                    # and is folded in afterwards by rescaling the AR payload
                    # with exp(m_loc - M)  (softmax-invariant).
                    mx_loc = small.tile([1, 16], f32, name=f"mxl{rep}_{it}")
                    nc.vector.memset(mx_loc[:], -1e30)
                    nc.vector.tensor_reduce(mx_loc[0:1, 0:C], b_sb[0:1, :, :],
                                            mybir.AxisListType.X,
                                            mybir.AluOpType.max)
                    mx_in = dram.tile([1, 16], f32, name=f"mxi{rep}_{it}")
                    mx_out = dram.tile([1, 16], f32, addr_space="Shared",
                                       name=f"mxo{rep}_{it}")
                    nc.sync.dma_start(mx_in[:], mx_loc[:])
                    if no_cc:
                        nc.sync.dma_start(mx_out[:, :], mx_in[:, :])
                    else:
                        nc.gpsimd.collective_compute(
                            "AllReduce", mybir.AluOpType.max, replica_groups=groups,
                            ins=[mx_in.opt()], outs=[mx_out.opt()])
                    nc.sync.dma_start(mx_row[:], mx_out[0:1, 0:C])
                    for c in range(C):
                        nc.vector.tensor_scalar(
                            b_shift[0:1, c, :], b_sb[0:1, c, :],
                            mx_loc[0:1, c:c + 1], None,
                            mybir.AluOpType.subtract)
                    if ws_parts < 2:
                        return
                    # w_sb = exp(b - m_loc) broadcast across partitions
                    b_flat = b_shift[:].rearrange("p c r -> p (c r)")
                    w_flat = w_sb[:].rearrange("p c r -> p (c r)")
                    off = 0
                    while off < C * r_loc:
                        n = min(512, C * r_loc - off)
                        wb = wbc_pool.tile([B, 512], f32, tag="wbc",
                                           name=f"wb{rep}_{it}_{off}")
                        nc.tensor.matmul(wb[:, 0:n], ones_sb[:], b_flat[:, off:off + n])
                        nc.scalar.activation(w_flat[:, off:off + n], wb[:, 0:n],
                                             mybir.ActivationFunctionType.Exp)
                        off += n
                    if ws_parts < 4:
                        return
                    # s numerator: fused mult+reduce per (o,c); d folded in as an
                    # extra all-ones STT per capsule
                    for c in range(C):
                        for o in range(O):
                            j = o * C + c
                            scr = scratch_pool.tile([B, r_loc], f32, tag="scr",
                                                    name=f"scr{rep}_{it}_{j}")
                            nc.vector.scalar_tensor_tensor(
                                scr[:], u_sb[:, o, c, :], 1.0, w_sb[:, c, :],
                                mybir.AluOpType.mult, mybir.AluOpType.mult,
                                accum_out=s_sb[:, j:j + 1])
                        scr = scratch_pool.tile([B, r_loc], f32, tag="scr",
                                                name=f"scrd{rep}_{it}_{c}")
                        nc.vector.scalar_tensor_tensor(
                            scr[:], ones_r[:], 1.0, w_sb[:, c, :],
                            mybir.AluOpType.mult, mybir.AluOpType.mult,
                            accum_out=d_all[:, c:c + 1])
                    # rescale payload by exp(m_loc - M); waits on the max-AR,
                    # which has been overlapping the work above
                    sc_row = small.tile([1, C], f32, name=f"sc{rep}_{it}")
                    nc.vector.tensor_tensor(sc_row[:], mx_loc[0:1, 0:C], mx_row[:],
                                            mybir.AluOpType.subtract)
                    nc.scalar.activation(sc_row[:], sc_row[:],
                                         mybir.ActivationFunctionType.Exp)
                    scps = wbc_pool.tile([B, 512], f32, tag="wbc",
                                         name=f"scps{rep}_{it}")
                    nc.tensor.matmul(scps[:, 0:C], ones_sb[:], sc_row[:])
                    for o in range(O):
                        nc.vector.tensor_tensor(
                            s_sb[:, o * C:(o + 1) * C], s_sb[:, o * C:(o + 1) * C],
                            scps[:, 0:C], mybir.AluOpType.mult)
                    nc.vector.tensor_tensor(d_all[:], d_all[:], scps[:, 0:C],
                                            mybir.AluOpType.mult)

                # ---------- iteration 0 ----------
                if stages >= 1:
                    all_reduce(s0_acc, with_d=False, it=0)
                    squash()
                else:
                    nc.vector.tensor_copy(v_sb[:], s0_acc[:])
                if stages >= 2:
                    agreement(0)
                # ---------- iteration 1 ----------
                if stages >= 3:
                    weights_and_s(1)
                if stages >= 4:
                    all_reduce(s_sb, with_d=True, it=1)
                    squash()
                if stages >= 5:
                    agreement(1)
                # ---------- iteration 2 ----------
                if stages >= 6:
                    weights_and_s(2)
                    all_reduce(s_sb, with_d=True, it=2)
                    squash()
                # ---------- output: reorder (o,c) -> (c,o) and store ----------
                nc.vector.tensor_copy(
                    out_sb[:].rearrange("p (c o) -> p c o", c=C),
                    v_sb[:].rearrange("p (o c) -> p c o", o=O))
                nc.sync.dma_start(out[:, :], out_sb[:])

    nc.compile()
    return nc


def _make_runner(nc):
    import jax
    from jax.sharding import Mesh, PartitionSpec, NamedSharding
    from jax.experimental.shard_map import shard_map
    from concourse import bass2jax, mybir
    from concourse.bass2jax import _bass_exec_p
    from concourse.mybir import MemoryLocationSet

    bass2jax.install_neuronx_cc_hook()
    partition_name = nc.partition_id_tensor.name if nc.partition_id_tensor else None
    in_names, out_names, out_avals, zero_outs = [], [], [], []
    for alloc in nc.m.functions[0].allocations:
        if not isinstance(alloc, MemoryLocationSet):
            continue
        name = alloc.memorylocations[0].name
        if alloc.kind == "ExternalInput":
            if name != partition_name:
                in_names.append(name)
        elif alloc.kind == "ExternalOutput":
            out_names.append(name)
            shape = tuple(alloc.tensor_shape)
            dtype = mybir.dt.np(alloc.dtype)
            out_avals.append(jax.core.ShapedArray(shape, dtype))
            zero_outs.append(np.zeros(shape, dtype))
    n_params = len(in_names)
    all_in_names = list(in_names) + out_names
    if partition_name is not None:
        all_in_names.append(partition_name)

    def _body(*args):
        operands = list(args)
        if partition_name is not None:
            operands.append(bass2jax.partition_id_tensor())
        outs = _bass_exec_p.bind(
            *operands, out_avals=tuple(out_avals), in_names=tuple(all_in_names),
            out_names=tuple(out_names), lowering_input_output_aliases=(),
            sim_require_finite=True, sim_require_nnan=True, nc=nc)
        return tuple(outs)

    devices = jax.devices()[:N_CORES]
    mesh = Mesh(np.asarray(devices), ("core",))
    in_specs = (PartitionSpec("core"),) * (n_params + len(out_names))
    out_specs = (PartitionSpec("core"),) * len(out_names)
    sharded = jax.jit(
        shard_map(_body, mesh=mesh, in_specs=in_specs, out_specs=out_specs,
                  check_rep=False),
        keep_unused=True)
    sharding = NamedSharding(mesh, PartitionSpec("core"))

    class Runner:
        _sharded = staticmethod(sharded)

        def put(self, in_maps):
            import jax as _jax
            concat = [np.concatenate([np.asarray(in_maps[c][nm])
                                      for c in range(N_CORES)], axis=0)
                      for nm in in_names]
            dz = [_jax.device_put(
                np.zeros((N_CORES * z.shape[0], *z.shape[1:]), z.dtype), sharding)
                for z in zero_outs]
            return [_jax.device_put(a, sharding) for a in concat] + dz

        def run(self, dev_args):
            import jax as _jax
            outs = sharded(*dev_args)
            _jax.block_until_ready(outs)
            return outs

        def results(self, outs):
            return [{nm: np.asarray(outs[i]).reshape(N_CORES, *out_avals[i].shape)[c]
                     for i, nm in enumerate(out_names)}
                    for c in range(N_CORES)]

    return Runner()


def _prep_shards(x, W, dma_group=1, host_layout=1):
    """Full inputs -> per-core in_maps with device-friendly layouts."""
    x = np.asarray(x, dtype=np.float32)
    W = np.asarray(W, dtype=np.float32)
    in_maps = []
    for k in range(N_CORES):
        rs = slice(k * R_LOC, (k + 1) * R_LOC)
        # xk: [r, i, b] padded i->384, viewed [r, ch, 128, b]
        xs = np.zeros((R_LOC, IP, B), dtype=np.float32)
        xs[:, :I, :] = np.transpose(x[:, rs, :], (1, 2, 0))
        # wk: [r, i, (o,c)] padded, viewed [r, ch, 128, 160]
        ws = np.zeros((R_LOC, IP, CO), dtype=np.float32)
        ws[:, :I, :] = np.transpose(W[rs], (0, 3, 2, 1)).reshape(R_LOC, I, CO)
        if host_layout == 3:
            G = dma_group
            xs = xs[:, :I, :]          # unpadded [r, i, b]
            ws = ws[:, :I, :]
            xg = xs.reshape(R_LOC // G, G, I, B)
            wg = ws.reshape(R_LOC // G, G, I, CO)
            xparts, wparts = [], []
            i0 = 0
            for K in CHUNKS:
                xparts.append(xg[:, :, i0:i0 + K, :].transpose(0, 2, 1, 3)
                              .reshape(R_LOC // G, -1))
                wparts.append(wg[:, :, i0:i0 + K, :].transpose(0, 2, 1, 3)
                              .reshape(R_LOC // G, -1))
                i0 += K
            xs = np.concatenate(xparts, axis=1).ravel()
            ws = np.concatenate(wparts, axis=1).ravel()
        else:
            xs = xs.reshape(R_LOC, NCH, 128, B)
            ws = ws.reshape(R_LOC, NCH, 128, CO)
            if host_layout == 2:
                G = dma_group
                xs = xs.reshape(R_LOC // G, G, NCH, 128, B).transpose(0, 3, 1, 2, 4)
                ws = ws.reshape(R_LOC // G, G, NCH, 128, CO).transpose(0, 3, 1, 2, 4)
        in_maps.append({
            "xk": np.ascontiguousarray(xs),
            "wk": np.ascontiguousarray(ws),
        })
    return in_maps


def _get_state():
    if "runner" not in _CACHE:
        nc = _build_nc()
        _CACHE["nc"] = nc
        _CACHE["runner"] = _make_runner(nc)
    return _CACHE["runner"]


def kernel(x, W):
    runner = _get_state()
    in_maps = _prep_shards(x, W)
    dev_args = runner.put(in_maps)
    outs = runner.run(dev_args)
    res = runner.results(outs)
    v = res[0]["out"]                       # [B, (c,o)]
    return v.reshape(B, C, O, 1)



# revision 12
# speedup vs baseline: 220.7962x; 220.7962x over previous
"""DigitCaps dynamic-routing kernel for 8 Trainium2 NeuronCores.

Problem: nn_DigitCaps  (B=128, R=1152, C=10, O=16, I=338, 3 routing iters)

    u_hat[b,r,c,o] = sum_i W[r,c,o,i] * x[b,r,i]
    b_ij = 0
    for it in 3:
        c_ij = softmax(b_ij, axis=routes)
        s[b,c,o] = sum_r c_ij[r,c] * u_hat[b,r,c,o]
        v = squash(s) = s*|s|/(1+s^2)
        if it < 2: b_ij += mean_b sum_o u_hat*v

Sharding: routes (R) split 8 ways -> 144 routes/core.  Each core computes
u_hat for its routes (PE matmuls, batch=128 on partitions), keeps it fully
SBUF-resident, and runs the routing iterations locally.  The only cross-core
communication is one fused AllReduce per iteration carrying the partial
softmax-weighted sum s (numerator) and the softmax denominator d.

Host-side prep (inside kernel()): per-core shards are transposed so both
matmul operands arrive with the contraction dim (i) on SBUF partitions:
  xk[r, i, b]   (i padded 338->384)   lhsT tiles [i=128, b=128]
  wk[r, i, o, c] (i padded 338->384)  rhs  tiles [i=128, (o,c)=160]
"""

import numpy as np

B, R, C, O, I = 128, 1152, 10, 16, 338
N_CORES = 8
R_LOC = R // N_CORES          # 144
IP = 384                      # i padded to 3*128
NCH = IP // 128               # 3 contraction chunks
CO = C * O                    # 160
CHUNKS = (128, 128, 82)       # unpadded i-chunk sizes (sum = I)
F32 = None                    # set lazily (mybir.dt.float32)

# Production kernel configuration (shared by kernel() and test.py's
# timing probes).  PREP_KEYS are the subset that _prep_shards needs.
#   host_layout=2/dma_group=4: one contiguous-per-partition DMA per tensor
#     per 4-route group (72 big DMAs instead of 288 small transposing ones).
#   nomax1: iteration 1 skips the softmax max-subtraction + its cross-core
#     max-AllReduce entirely -- |b_1| <= 4*rms(u)*||v|| ~ 74 < 88 = ln(f32
#     max), so exp(b_1) cannot overflow (measured max|b_1| = 11.3).
#     Iteration 2 keeps it (max|b_2| = 231 would overflow).
PROD_CFG = dict(host_layout=2, dma_group=4, nomax1=True)
PREP_KEYS = ("dma_group", "host_layout", "dt16")


def _prep_cfg():
    return {k: v for k, v in PROD_CFG.items() if k in PREP_KEYS}


_CACHE = {}


def _build_nc(r_loc=R_LOC, n_cores=N_CORES, stages=99, ws_parts=4, reps=1,
              stream_bufs=6, dma_group=1, s0_late=False, host_layout=1,
              no_cc=False, spass_split=False, dt16=False, nomax1=False):
    import concourse.tile as tile
    from concourse import bacc, mybir

    f32 = mybir.dt.float32
    fin = mybir.dt.float16 if dt16 else f32
    nc = bacc.Bacc("TRN2", target_bir_lowering=False, debug=False,
                   enable_asserts=False, num_devices=n_cores)

    if host_layout == 3:
        ng = r_loc // dma_group
        xk = nc.dram_tensor("xk", [ng * I * dma_group * B], fin,
                            kind="ExternalInput")
        wk = nc.dram_tensor("wk", [ng * I * dma_group * CO], fin,
                            kind="ExternalInput")
    elif host_layout == 2:
        xk = nc.dram_tensor("xk", [r_loc // dma_group, 128, dma_group, NCH, B],
                            f32, kind="ExternalInput")
        wk = nc.dram_tensor("wk", [r_loc // dma_group, 128, dma_group, NCH, CO],
                            f32, kind="ExternalInput")
    else:
        xk = nc.dram_tensor("xk", [r_loc, NCH, 128, B], f32, kind="ExternalInput")
        wk = nc.dram_tensor("wk", [r_loc, NCH, 128, CO], f32, kind="ExternalInput")
    out = nc.dram_tensor("out", [B, CO], f32, kind="ExternalOutput")

    groups = [list(range(n_cores))]

    with tile.TileContext(nc) as tc:
        with (
            tc.tile_pool(name="u", bufs=1) as u_pool,
            tc.tile_pool(name="stream", bufs=stream_bufs) as stream,
            tc.tile_pool(name="small", bufs=1) as small,
            tc.tile_pool(name="scratch", bufs=4) as scratch_pool,
            tc.tile_pool(name="upsum", bufs=4, space="PSUM") as upsum_pool,
            tc.tile_pool(name="apsum", bufs=2, space="PSUM") as apsum_pool,
            tc.tile_pool(name="wbc", bufs=2, space="PSUM") as wbc_pool,
            tc.tile_pool(name="dram", bufs=1, space="DRAM") as dram,
        ):
            # u_hat resident tile: [b=128, o, c, r]
            u_sb = u_pool.tile([B, O, C, r_loc], f32)
            s0_acc = small.tile([B, CO], f32)       # (o,c) flat
            nc.vector.memset(s0_acc[:], 0.0)
            ones_sb = small.tile([1, 128], f32)
            nc.vector.memset(ones_sb[:], 1.0)
            b_sb = small.tile([1, C, r_loc], f32)
            nc.vector.memset(b_sb[:], 0.0)
            b_shift = small.tile([1, C, r_loc], f32)
            ones_r = small.tile([B, r_loc], f32)
            nc.vector.memset(ones_r[:], 1.0)
            w_sb = small.tile([B, C, r_loc], f32)   # exp(b) broadcast to all b
            s_sb = small.tile([B, CO], f32)         # s numerator / s (o,c) flat
            v_sb = small.tile([B, CO], f32)
            d_all = small.tile([B, C], f32)
            rd_sb = small.tile([B, C], f32)
            d_row = small.tile([1, CO], f32)
            mx_row = small.tile([1, C], f32)
            sq_t = small.tile([B, CO], f32)
            den_t = small.tile([B, CO], f32)
            rden_t = small.tile([B, CO], f32)
            sabs_t = small.tile([B, CO], f32)
            st_t = small.tile([B, CO], f32)
            out_sb = small.tile([B, CO], f32)

            for rep in range(reps):
                if rep > 0:
                    nc.vector.memset(s0_acc[:], 0.0)
                    nc.vector.memset(b_sb[:], 0.0)
                # ---------- Phase A: u_hat = x @ W per route ----------
                G = dma_group
                assert r_loc % G == 0
                for rg in range(r_loc // G):
                    x_t = stream.tile([128, G, NCH, B], fin, tag="x")
                    w_t = stream.tile([128, G, NCH, CO], fin, tag="w")
                    if host_layout == 3:
                        xoff = rg * I * G * B
                        woff = rg * I * G * CO
                        for ch, K in enumerate(CHUNKS):
                            nc.sync.dma_start(
                                x_t[0:K, :, ch, :],
                                xk[xoff:xoff + K * G * B].rearrange(
                                    "(p g b) -> p g b", p=K, g=G))
                            xoff += K * G * B
                            nc.sync.dma_start(
                                w_t[0:K, :, ch, :],
                                wk[woff:woff + K * G * CO].rearrange(
                                    "(p g n) -> p g n", p=K, g=G))
                            woff += K * G * CO
                    elif host_layout == 2:
                        nc.sync.dma_start(x_t[:], xk[rg])
                        nc.sync.dma_start(w_t[:], wk[rg])
                    else:
                        nc.sync.dma_start(
                            x_t[:], xk[rg * G:(rg + 1) * G].rearrange(
                                "g ch p b -> p g ch b"))
                        nc.sync.dma_start(
                            w_t[:], wk[rg * G:(rg + 1) * G].rearrange(
                                "g ch p n -> p g ch n"))
                    for g in range(G):
                        r = rg * G + g
                        upsum = upsum_pool.tile([B, CO], f32)
                        for ch in range(NCH):
                            K = CHUNKS[ch] if host_layout == 3 else 128
                            nc.tensor.matmul(upsum[:], x_t[0:K, g, ch, :],
                                             w_t[0:K, g, ch, :],
                                             start=(ch == 0), stop=(ch == NCH - 1))
                        # copy into resident u_hat ([b,(o,c)] -> [b,o,c,r])
                        nc.scalar.activation(
                            u_sb[:, :, :, r],
                            upsum[:].rearrange("p (o c) -> p o c", o=O),
                            mybir.ActivationFunctionType.Copy)
                        if not s0_late:
                            nc.vector.tensor_tensor(s0_acc[:], s0_acc[:], upsum[:],
                                                    mybir.AluOpType.add)
                if s0_late:
                    # s0 = sum_r u_hat via 16 reduces over the resident tile
                    for o in range(O):
                        nc.vector.tensor_reduce(
                            s0_acc[:, o * C:(o + 1) * C], u_sb[:, o, :, :],
                            mybir.AxisListType.X, mybir.AluOpType.add)

                # ---------- helpers ----------
                def all_reduce(sb_src, with_d, it):
                    rows = B + 1 if with_d else B
                    ar_in = dram.tile([rows, CO], f32, name=f"ar_in{rep}_{it}")
                    ar_out = dram.tile([rows, CO], f32, addr_space="Shared",
                                       name=f"ar_out{rep}_{it}")
                    nc.sync.dma_start(ar_in[0:B, :], sb_src[:])
                    if with_d:
                        nc.sync.dma_start(ar_in[B:B + 1, 0:C], d_all[0:1, :])
                    if no_cc:
                        nc.sync.dma_start(ar_out[:, :], ar_in[:, :])
                    else:
                        nc.gpsimd.collective_compute(
                            "AllReduce", mybir.AluOpType.add, replica_groups=groups,
                            ins=[ar_in.opt()], outs=[ar_out.opt()])
                    nc.sync.dma_start(s_sb[:], ar_out[0:B, :])
                    if with_d:
                        d2 = small.tile([1, C], f32, name=f"d2_{rep}_{it}")
                        nc.sync.dma_start(d2[:], ar_out[B:B + 1, 0:C])
                        dps = wbc_pool.tile([B, 512], f32, tag="wbc", name=f"dps{rep}_{it}")
                        nc.tensor.matmul(dps[:, 0:C], ones_sb[:], d2[:])
                        nc.vector.reciprocal(rd_sb[:], dps[:, 0:C])
                        for o in range(O):
                            nc.vector.tensor_tensor(
                                s_sb[:, o * C:(o + 1) * C], s_sb[:, o * C:(o + 1) * C],
                                rd_sb[:], mybir.AluOpType.mult)
                    else:
                        nc.vector.tensor_scalar_mul(s_sb[:], s_sb[:], 1.0 / (r_loc * n_cores))

                def squash():
                    # v = s*|s| / (1+s^2)
                    nc.vector.tensor_tensor(sq_t[:], s_sb[:], s_sb[:],
                                            mybir.AluOpType.mult)
                    nc.vector.tensor_scalar_add(den_t[:], sq_t[:], 1.0)
                    nc.vector.reciprocal(rden_t[:], den_t[:])
                    nc.scalar.activation(sabs_t[:], s_sb[:],
                                         mybir.ActivationFunctionType.Abs)
                    nc.vector.tensor_tensor(st_t[:], s_sb[:], sabs_t[:],
                                            mybir.AluOpType.mult)
                    nc.vector.tensor_tensor(v_sb[:], st_t[:], rden_t[:],
                                            mybir.AluOpType.mult)

                def agreement(it):
                    # b += (1/B) * sum_b sum_o u_hat * v   (PE, K=b contraction)
                    for c in range(C):
                        aps = apsum_pool.tile([1, r_loc], f32, tag="a", name=f"a{rep}_{it}_{c}")
                        for o in range(O):
                            j = o * C + c
                            nc.tensor.matmul(aps[:], v_sb[:, j:j + 1], u_sb[:, o, c, :],
                                             start=(o == 0), stop=(o == O - 1))
                        nc.vector.scalar_tensor_tensor(
                            b_sb[0:1, c, :], aps[:], 1.0 / B, b_sb[0:1, c, :],
                            mybir.AluOpType.mult, mybir.AluOpType.add)

                def weights_and_s(it):
                    skip_max = nomax1 and it == 1
                    if ws_parts < 4:
                        # truncated profiling builds: keep AR operands written
                        nc.vector.memset(s_sb[:], 0.0)
                        nc.vector.memset(w_sb[:], 0.0)
                        nc.vector.memset(d_all[:], 0.0)
                    if ws_parts < 1:
                        return
                    # LOCAL softmax max-subtraction; the cross-core max arrives
                    # via a small AllReduce(max) that overlaps exp + the s-pass,
                    # and is folded in afterwards by rescaling the AR payload
                    # with exp(m_loc - M)  (softmax-invariant).
                    # Iteration 1 can skip all of it (nomax1): |b1| <=
                    # 4*rms(u)*||v|| ~ 74 < 88, so exp(b1) cannot overflow f32.
                    if not skip_max:
                        mx_loc = small.tile([1, 16], f32, name=f"mxl{rep}_{it}")
                        nc.vector.memset(mx_loc[:], -1e30)
                        nc.vector.tensor_reduce(mx_loc[0:1, 0:C], b_sb[0:1, :, :],
                                                mybir.AxisListType.X,
                                                mybir.AluOpType.max)
                        mx_in = dram.tile([1, 16], f32, name=f"mxi{rep}_{it}")
                        mx_out = dram.tile([1, 16], f32, addr_space="Shared",
                                           name=f"mxo{rep}_{it}")
                        nc.sync.dma_start(mx_in[:], mx_loc[:])
                        if no_cc:
                            nc.sync.dma_start(mx_out[:, :], mx_in[:, :])
                        else:
                            nc.gpsimd.collective_compute(
                                "AllReduce", mybir.AluOpType.max, replica_groups=groups,
                                ins=[mx_in.opt()], outs=[mx_out.opt()])
                        nc.sync.dma_start(mx_row[:], mx_out[0:1, 0:C])
                        for c in range(C):
                            nc.vector.tensor_scalar(
                                b_shift[0:1, c, :], b_sb[0:1, c, :],
                                mx_loc[0:1, c:c + 1], None,
                                mybir.AluOpType.subtract)
                    if ws_parts < 2:
                        return
                    # w_sb = exp(b - m_loc) broadcast across partitions
                    b_flat = (b_sb if skip_max else b_shift)[:].rearrange(
                        "p c r -> p (c r)")
                    w_flat = w_sb[:].rearrange("p c r -> p (c r)")
                    off = 0
                    while off < C * r_loc:
                        n = min(512, C * r_loc - off)
                        wb = wbc_pool.tile([B, 512], f32, tag="wbc",
                                           name=f"wb{rep}_{it}_{off}")
                        nc.tensor.matmul(wb[:, 0:n], ones_sb[:], b_flat[:, off:off + n])
                        nc.scalar.activation(w_flat[:, off:off + n], wb[:, 0:n],
                                             mybir.ActivationFunctionType.Exp)
                        off += n
                    if ws_parts < 4:
                        return
                    # s numerator: fused mult+reduce per (o,c); d folded in as an
                    # extra all-ones STT per capsule
                    for c in range(C):
                        for o in range(O):
                            j = o * C + c
                            scr = scratch_pool.tile([B, r_loc], f32, tag="scr",
                                                    name=f"scr{rep}_{it}_{j}")
                            nc.vector.scalar_tensor_tensor(
                                scr[:], u_sb[:, o, c, :], 1.0, w_sb[:, c, :],
                                mybir.AluOpType.mult, mybir.AluOpType.mult,
                                accum_out=s_sb[:, j:j + 1])
                        scr = scratch_pool.tile([B, r_loc], f32, tag="scr",
                                                name=f"scrd{rep}_{it}_{c}")
                        nc.vector.scalar_tensor_tensor(
                            scr[:], ones_r[:], 1.0, w_sb[:, c, :],
                            mybir.AluOpType.mult, mybir.AluOpType.mult,
                            accum_out=d_all[:, c:c + 1])
                    if skip_max:
                        return
                    # rescale payload by exp(m_loc - M); waits on the max-AR,
                    # which has been overlapping the work above
                    sc_row = small.tile([1, C], f32, name=f"sc{rep}_{it}")
                    nc.vector.tensor_tensor(sc_row[:], mx_loc[0:1, 0:C], mx_row[:],
                                            mybir.AluOpType.subtract)
                    nc.scalar.activation(sc_row[:], sc_row[:],
                                         mybir.ActivationFunctionType.Exp)
                    scps = wbc_pool.tile([B, 512], f32, tag="wbc",
                                         name=f"scps{rep}_{it}")
                    nc.tensor.matmul(scps[:, 0:C], ones_sb[:], sc_row[:])
                    for o in range(O):
                        nc.vector.tensor_tensor(
                            s_sb[:, o * C:(o + 1) * C], s_sb[:, o * C:(o + 1) * C],
                            scps[:, 0:C], mybir.AluOpType.mult)
                    nc.vector.tensor_tensor(d_all[:], d_all[:], scps[:, 0:C],
                                            mybir.AluOpType.mult)

                # ---------- iteration 0 ----------
                if stages >= 1:
                    all_reduce(s0_acc, with_d=False, it=0)
                    squash()
                else:
                    nc.vector.tensor_copy(v_sb[:], s0_acc[:])
                if stages >= 2:
                    agreement(0)
                # ---------- iteration 1 ----------
                if stages >= 3:
                    weights_and_s(1)
                if stages >= 4:
                    all_reduce(s_sb, with_d=True, it=1)
                    squash()
                if stages >= 5:
                    agreement(1)
                # ---------- iteration 2 ----------
                if stages >= 6:
                    weights_and_s(2)
                    all_reduce(s_sb, with_d=True, it=2)
                    squash()
                # ---------- output: reorder (o,c) -> (c,o) and store ----------
                nc.vector.tensor_copy(
                    out_sb[:].rearrange("p (c o) -> p c o", c=C),
                    v_sb[:].rearrange("p (o c) -> p c o", o=O))
                nc.sync.dma_start(out[:, :], out_sb[:])

    nc.compile()
    return nc


def _make_runner(nc):
    import jax
    from jax.sharding import Mesh, PartitionSpec, NamedSharding
    from jax.experimental.shard_map import shard_map
    from concourse import bass2jax, mybir
    from concourse.bass2jax import _bass_exec_p
    from concourse.mybir import MemoryLocationSet

    bass2jax.install_neuronx_cc_hook()
    partition_name = nc.partition_id_tensor.name if nc.partition_id_tensor else None
    in_names, out_names, out_avals, zero_outs = [], [], [], []
    for alloc in nc.m.functions[0].allocations:
        if not isinstance(alloc, MemoryLocationSet):
            continue
        name = alloc.memorylocations[0].name
        if alloc.kind == "ExternalInput":
            if name != partition_name:
                in_names.append(name)
        elif alloc.kind == "ExternalOutput":
            out_names.append(name)
            shape = tuple(alloc.tensor_shape)
            dtype = mybir.dt.np(alloc.dtype)
            out_avals.append(jax.core.ShapedArray(shape, dtype))
            zero_outs.append(np.zeros(shape, dtype))
    n_params = len(in_names)
    all_in_names = list(in_names) + out_names
    if partition_name is not None:
        all_in_names.append(partition_name)

    def _body(*args):
        operands = list(args)
        if partition_name is not None:
            operands.append(bass2jax.partition_id_tensor())
        outs = _bass_exec_p.bind(
            *operands, out_avals=tuple(out_avals), in_names=tuple(all_in_names),
            out_names=tuple(out_names), lowering_input_output_aliases=(),
            sim_require_finite=True, sim_require_nnan=True, nc=nc)
        return tuple(outs)

    devices = jax.devices()[:N_CORES]
    mesh = Mesh(np.asarray(devices), ("core",))
    in_specs = (PartitionSpec("core"),) * (n_params + len(out_names))
    out_specs = (PartitionSpec("core"),) * len(out_names)
    sharding = NamedSharding(mesh, PartitionSpec("core"))

    # fast-dispatch compile: suppresses the BassEffect so per-call dispatch
    # takes JAX's C++ fast path instead of the effectful python path.
    shapes = {}
    for alloc in nc.m.functions[0].allocations:
        if not isinstance(alloc, MemoryLocationSet):
            continue
        name = alloc.memorylocations[0].name
        if alloc.kind in ("ExternalInput", "ExternalOutput"):
            shapes[name] = (tuple(alloc.tensor_shape), mybir.dt.np(alloc.dtype))
    in_avals = [jax.ShapeDtypeStruct(
        (N_CORES * shapes[nm][0][0], *shapes[nm][0][1:]), shapes[nm][1],
        sharding=sharding) for nm in in_names + out_names]

    def _compile_fn():
        return jax.jit(
            shard_map(_body, mesh=mesh, in_specs=in_specs, out_specs=out_specs,
                      check_rep=False),
            keep_unused=True).lower(*in_avals).compile()

    sharded = bass2jax.fast_dispatch_compile(_compile_fn)

    class Runner:
        _sharded = staticmethod(sharded)

        def put(self, in_maps):
            import jax as _jax
            concat = [np.concatenate([np.asarray(in_maps[c][nm])
                                      for c in range(N_CORES)], axis=0)
                      for nm in in_names]
            dz = [_jax.device_put(
                np.zeros((N_CORES * z.shape[0], *z.shape[1:]), z.dtype), sharding)
                for z in zero_outs]
            return [_jax.device_put(a, sharding) for a in concat] + dz

        def run(self, dev_args):
            import jax as _jax
            outs = sharded(*dev_args)
            _jax.block_until_ready(outs)
            return outs

        def results(self, outs):
            return [{nm: np.asarray(outs[i]).reshape(N_CORES, *out_avals[i].shape)[c]
                     for i, nm in enumerate(out_names)}
                    for c in range(N_CORES)]

    return Runner()


def _prep_shards(x, W, dma_group=1, host_layout=1, dt16=False):
    """Full inputs -> per-core in_maps with device-friendly layouts."""
    dt = np.float16 if dt16 else np.float32
    x = np.asarray(x, dtype=np.float32)
    W = np.asarray(W, dtype=np.float32)
    in_maps = []
    for k in range(N_CORES):
        rs = slice(k * R_LOC, (k + 1) * R_LOC)
        # xk: [r, i, b] padded i->384, viewed [r, ch, 128, b]
        xs = np.zeros((R_LOC, IP, B), dtype=np.float32)
        xs[:, :I, :] = np.transpose(x[:, rs, :], (1, 2, 0))
        # wk: [r, i, (o,c)] padded, viewed [r, ch, 128, 160]
        ws = np.zeros((R_LOC, IP, CO), dtype=np.float32)
        ws[:, :I, :] = np.transpose(W[rs], (0, 3, 2, 1)).reshape(R_LOC, I, CO)
        if host_layout == 3:
            G = dma_group
            xs = xs[:, :I, :]          # unpadded [r, i, b]
            ws = ws[:, :I, :]
            xg = xs.reshape(R_LOC // G, G, I, B)
            wg = ws.reshape(R_LOC // G, G, I, CO)
            xparts, wparts = [], []
            i0 = 0
            for K in CHUNKS:
                xparts.append(xg[:, :, i0:i0 + K, :].transpose(0, 2, 1, 3)
                              .reshape(R_LOC // G, -1))
                wparts.append(wg[:, :, i0:i0 + K, :].transpose(0, 2, 1, 3)
                              .reshape(R_LOC // G, -1))
                i0 += K
            xs = np.concatenate(xparts, axis=1).astype(dt).ravel()
            ws = np.concatenate(wparts, axis=1).astype(dt).ravel()
        else:
            xs = xs.reshape(R_LOC, NCH, 128, B)
            ws = ws.reshape(R_LOC, NCH, 128, CO)
            if host_layout == 2:
                G = dma_group
                xs = xs.reshape(R_LOC // G, G, NCH, 128, B).transpose(0, 3, 1, 2, 4)
                ws = ws.reshape(R_LOC // G, G, NCH, 128, CO).transpose(0, 3, 1, 2, 4)
        in_maps.append({
            "xk": np.ascontiguousarray(xs),
            "wk": np.ascontiguousarray(ws),
        })
    return in_maps


def _get_state():
    if "runner" not in _CACHE:
        nc = _build_nc(**PROD_CFG)
        _CACHE["nc"] = nc
        _CACHE["runner"] = _make_runner(nc)
    return _CACHE["runner"]


def kernel(x, W):
    runner = _get_state()
    in_maps = _prep_shards(x, W, **_prep_cfg())
    dev_args = runner.put(in_maps)
    outs = runner.run(dev_args)
    res = runner.results(outs)
    v = res[0]["out"]                       # [B, (c,o)]
    return v.reshape(B, C, O, 1)

